# revision 35
# baseline (speedup 1.0000x reference)
"""Trainium2 Bass kernel for leave-one-out Nadaraya-Watson regression
(nn_Net_7610682049228, retrieval_knn).

Math
----
Zw = relu(x @ W1.T) @ W2.T          [N, 3]
Xw = relu(train_X @ W1.T) @ W2.T    [N, 3]
K[i,j,d] = exp(-((Xw[j,d]-Zw[i,d])/h)^2 / 2), diagonal i==j masked out
out[i,d] = sum_j K*Y / sum_j K

Kernel factorization (the key trick):
  K[i,j,d] = G[j,d] * H[i,d] * C[i,j,d]
    G[j,d] = exp(-Xw[j,d]^2 / 2h^2)        (O(N) precompute)
    H[i,d] = exp(-Zw[i,d]^2 / 2h^2)        (cancels in the ratio!)
    C[i,j,d] = exp(Zw[i,d]*Xw[j,d] / h^2)  (rank-1 exponent)
  out[i,d] = (sum_j C*G*Y - c_i*Y_i) / (sum_j C*G - c_i)
    with the leave-one-out correction c[i,d] = exp((Zw*Xw - Xw^2/2)/h^2)|_{j=i}.

So the only O(N^2) work is: a rank-1 outer product (DVE tensor_scalar with a
per-partition scalar), one big Exp pass (ACT engine - the throughput floor),
and [G*Y | G]-weighted column reductions (PE matmuls accumulating in PSUM).

Sharding: data-parallel over query rows i; core m handles i in
[512m, 512m+512). j lives on SBUF partitions (32 blocks of 128), the 512
i-columns of the shard live on the free dim. No cross-core communication.

All input-dependent scalars (h, W2) are consumed as tensors, so the compiled
program is input-independent and built/compiled once per process.

Execution layer: on this axon-tunneled setup every device roundtrip has a
fixed ~70-85ms turnaround (payload- and device-count-insensitive; the
request leaves in ~2ms and the line is silent until the response), so the
warm-call wall time is one roundtrip plus host packing. kernel() therefore
AOT-compiles one fast-dispatch executable for the 8-core shard_map program
(cached in _CACHE) and each call is: pack inputs (~1ms) -> dispatch
(~2-9ms, upload overlaps the roundtrip) -> one await+fetch (~70ms).
run_bass_kernel_spmd, which rebuilds its jax.jit closure per call
(~200ms retrace), is kept as the trace/debug and fallback path.
"""

import numpy as np
from contextlib import ExitStack

import concourse.bacc as bacc
import concourse.bass as bass
import concourse.mybir as mybir
import concourse.tile as tile
from concourse.bass_utils import run_bass_kernel_spmd

F32 = mybir.dt.float32
AF = mybir.ActivationFunctionType
OP = mybir.AluOpType

N = 4096
NCORES = 8
SHARD = N // NCORES          # 512 query rows per core
P = 128                      # SBUF partitions
JB = N // P                  # 32 j-blocks
D = 3                        # output dims
JB_PER_CHUNK = 2             # j-blocks fused into one ACT Exp instruction
NCHUNK = JB // JB_PER_CHUNK  # 16
CHUNK_W = JB_PER_CHUNK * D * SHARD  # 3072 free elements per chunk
C4_W = SHARD + D + N + SHARD           # [xTs | W1T | tXT | tXTs]
C3_W = D + SHARD + D * P + D * D + 1   # [W2T | YTs | sel | W2f | h]

_CACHE = {}


def _build_program(reps: int = 0, parts: str = "tem", cdt: str = "r", cjb: int = JB_PER_CHUNK) -> bass.Bass:
    # Bacc (not raw Bass): its compile() pass legalizes multi-wait
    # instructions for walrus, which allows only 1-2 sync waits per op.
    # reps > 0 wraps the main O(N^2) loop in a hardware For_i that repeats it
    # `reps` times — used only for wall-clock calibration benchmarks.
    # parts: which main-loop stages to emit (t=tensor_scalar, e=exp, m=matmul)
    # — benchmarking aid, always "tem" for real runs.
    nc = bacc.Bacc("TRN2", target_bir_lowering=False, debug=False)

    # --- DRAM I/O (per-core shapes; host preps layouts/slices) ---
    d_c4 = nc.dram_tensor("c4", (4, C4_W), F32, kind="ExternalInput").ap()
    d_c3 = nc.dram_tensor("c3", (D, C3_W), F32, kind="ExternalInput").ap()
    d_Yj = nc.dram_tensor("Yj", (P, JB * D), F32, kind="ExternalInput").ap()
    d_outT = nc.dram_tensor("outT", (D, SHARD), F32, kind="ExternalOutput").ap()

    with tile.TileContext(nc) as tc, ExitStack() as ctx:
        sb = ctx.enter_context(tc.tile_pool(name="sb", bufs=1))
        pp = ctx.enter_context(tc.tile_pool(name="pp", bufs=3))
        cp = ctx.enter_context(tc.tile_pool(name="cp", bufs=3))
        ps = ctx.enter_context(tc.tile_pool(name="ps", bufs=1, space="PSUM"))
        pr = ctx.enter_context(tc.tile_pool(name="pr", bufs=1, space="PSUM"))
        # Setup matmul chains get their own PSUM tiles so the chains overlap
        # (one shared scratch serialized the whole ~23us setup through
        # PE->copy->PE round trips). 8 banks: z, x, bj, r0, r1 + 3x red.
        PS_z = ps.tile([D, SHARD], F32, tag="ps_z", name="PS_z")
        PS_x = ps.tile([D, SHARD], F32, tag="ps_x", name="PS_x")
        PS_bj = ps.tile([P, JB * D], F32, tag="ps_bj", name="PS_bj")
        PS_b = ps.tile([P, 1 + D * D], F32, tag="ps_b", name="PS_b")

        # ---------- load inputs (HWDGE; Bacc legalizes multi-wait consumers)
        # Host packs the small tensors into combo blobs to minimize DMA
        # instruction count (each DMA costs ~descriptor-count in setup time).
        def load(dram_ap, shape, name):
            t = sb.tile(shape, F32, name=name)
            nc.sync.dma_start(t, dram_ap)
            return t

        # c4 split: the 8KB [xTs|W1T] head lands first so the query-MLP
        # (the long pole of setup: MM1_z -> relu -> MM2_z -> Zrep) isn't
        # gated on the 82KB tail. c3/Yj issue from the Activation HWDGE
        # queue so the four input DMAs overlap across both queues.
        HEAD = SHARD + D
        c4 = sb.tile([4, C4_W], F32, name="c4")
        nc.sync.dma_start(c4[:, 0:HEAD], d_c4[:, 0:HEAD])
        nc.sync.dma_start(c4[:, HEAD:], d_c4[:, HEAD:])
        xTs = c4[:, 0:SHARD]
        W1T = c4[:, SHARD : SHARD + D]
        tXT = c4[:, HEAD : HEAD + N]
        tXTs = c4[:, HEAD + N : HEAD + N + SHARD]
        c3 = sb.tile([D, C3_W], F32, name="c3")
        nc.scalar.dma_start(c3, d_c3)
        W2T = c3[:, 0:D]
        YTs = c3[:, D : D + SHARD]
        sel = c3[:, D + SHARD : D + SHARD + D * P]
        W2f = c3[0:1, D + SHARD + D * P : D + SHARD + D * P + D * D]
        h_sb = c3[0:1, D + SHARD + D * P + D * D : D + SHARD + D * P + D * D + 1]
        Yj = sb.tile([P, JB * D], F32, name="Yj")
        nc.scalar.dma_start(Yj, d_Yj)

        ones = sb.tile([1, P], F32)
        nc.vector.memset(ones, 1.0)
        zb = sb.tile([P, 1], F32)  # zero bias for activations
        nc.vector.memset(zb, 0.0)

        # ---------- broadcast scalars: 1/h^2 and W2 across partitions ----------
        hsq = sb.tile([1, 1], F32)
        nc.vector.tensor_mul(hsq, h_sb, h_sb)
        hinv = sb.tile([1, 1], F32)
        nc.vector.reciprocal(hinv, hsq)
        W2h = sb.tile([1, 1 + D * D], F32)  # [1/h^2, W2 row-major]
        nc.vector.tensor_copy(W2h[:, 0:1], hinv)
        nc.vector.tensor_copy(W2h[:, 1:], W2f)
        # broadcast gets its own PSUM tile so the bc copy doesn't create a
        # false WAR dependency that delays the j-MLP matmuls
        nc.tensor.matmul(PS_b, ones, W2h, start=True, stop=True)
        bc = sb.tile([P, 1 + D * D], F32)
        nc.vector.tensor_copy(bc, PS_b)
        invh2 = bc[:, 0:1]

        def w2col(d, m):  # W2[d,m] broadcast per-partition
            return bc[:, 1 + D * d + m : 2 + D * d + m]

        nh = sb.tile([P, 1], F32)  # -1/(2 h^2), ACT scale for G
        nc.vector.tensor_scalar_mul(nh, invh2, -0.5)

        # fp32r: PE streams it at 1 col/cycle when the moving dim >= 256
        # (plain fp32 matmul is 4x slower), at slightly reduced precision.
        # walrus requires fp32r matmul operands to be *produced* as fp32r,
        # so the hot-loop tiles (C, W6) are allocated fp32r and rounded on
        # write by ACT/DVE; the tiny setup matmuls stay plain fp32.
        F32R = mybir.dt.float32r

        # ---------- MLPs. Critical-path chains first: the query MLP
        # (-> ZwT -> Zrep) and the j-layout MLP (-> Xws/W6) gate the main
        # loop; the XwTs chain feeds only the epilogue and is emitted last.
        nc.tensor.matmul(PS_z, W1T, xTs, start=True, stop=True)
        for jb in range(JB):
            nc.tensor.matmul(
                PS_bj[:, D * jb : D * (jb + 1)],
                tXT[:, P * jb : P * (jb + 1)],
                W1T,
                start=True,
                stop=True,
            )
        hidz = sb.tile([D, SHARD], F32, name="hidz")
        nc.scalar.activation(hidz, PS_z, AF.Relu, bias=zb[0:D, :])
        h1j = sb.tile([P, JB * D], F32)
        nc.scalar.activation(h1j, PS_bj[:, 0 : JB * D], AF.Relu, bias=zb)
        nc.tensor.matmul(PS_z, W2T, hidz, start=True, stop=True)
        ZwT = sb.tile([D, SHARD], F32, name="mlpTz")  # unscaled
        # Identity-activation copy keeps this off the congested DVE queue
        nc.scalar.activation(ZwT, PS_z, AF.Copy, bias=0.0)
        # layer 2 on DVE with per-partition W2 scalars
        h1r = h1j.rearrange("p (a m) -> p a m", m=D)
        Xwj = sb.tile([P, JB * D], F32)
        Xwr = Xwj.rearrange("p (a d) -> p a d", d=D)
        for d in range(D):
            acc0 = sb.tile([P, JB], F32, tag="l2a", name="acc0")
            nc.vector.tensor_scalar_mul(acc0, h1r[:, :, 0], w2col(d, 0))
            acc1 = sb.tile([P, JB], F32, tag="l2b", name="acc1")
            nc.vector.scalar_tensor_tensor(
                acc1, h1r[:, :, 1], w2col(d, 1), acc0, OP.mult, OP.add
            )
            nc.vector.scalar_tensor_tensor(
                Xwr[:, :, d], h1r[:, :, 2], w2col(d, 2), acc1, OP.mult, OP.add
            )
        # Xw scaled by 1/h^2: the per-partition scalar for the rank-1 products
        Xws = sb.tile([P, JB * D], F32)
        nc.vector.tensor_scalar_mul(Xws, Xwj, invh2)

        # ---------- G, G*Y -> interleaved matmul weights W6 ----------
        sq = sb.tile([P, JB * D], F32)
        nc.vector.tensor_mul(sq, Xwj, Xwj)
        W6 = sb.tile(
            [P, JB * D * 2],
            {"r": F32R, "f": F32, "b": mybir.dt.bfloat16, "h": mybir.dt.float16}[cdt],
        )
        W6r = W6.rearrange("p (a t) -> p a t", t=2)
        # write G and G*Y straight into the interleaved weight tile (strided
        # outputs) instead of materializing Gj/GYj and copying them over
        nc.scalar.activation(W6r[:, :, 1], sq, AF.Exp, bias=zb, scale=nh)
        nc.vector.tensor_mul(W6r[:, :, 0], W6r[:, :, 1], Yj)

        # ---------- Zw replicated across partitions: [128, 3*512] ----------
        # Linearize ZwT [3,512] onto partition 0 with one SBUF->SBUF DMA,
        # then a single GpSimd partition_broadcast fans it out to all 128
        # partitions. This replaces three 2.4us contraction-3 PE matmuls
        # (the old one-hot sel trick) with ~2us on otherwise-idle engines.
        Zw3 = sb.tile([1, D * SHARD], F32, name="Zw3")
        nc.sync.dma_start(Zw3, ZwT)
        Zrep = sb.tile([P, D * SHARD], F32)
        nc.gpsimd.partition_broadcast(Zrep, Zw3)

        # deferred XwTs chain (feeds only the leave-one-out epilogue)
        nc.tensor.matmul(PS_x, W1T, tXTs, start=True, stop=True)
        hidx = sb.tile([D, SHARD], F32, name="hidx")
        nc.scalar.activation(hidx, PS_x, AF.Relu, bias=zb[0:D, :])
        nc.tensor.matmul(PS_x, W2T, hidx, start=True, stop=True)
        XwTs = sb.tile([D, SHARD], F32, name="mlpTx")  # unscaled
        nc.vector.tensor_copy(XwTs, PS_x)

        # ---------- main O(N^2) loop ----------
        red = [
            pr.tile([2, SHARD], F32, tag=f"red{d}", name=f"red{d}") for d in range(D)
        ]
        if "m" not in parts:  # bench-only: keep epilogue readers legal
            for d in range(D):
                nc.vector.memset(red[d], 1.0)
        n_chunk = JB // cjb
        chunk_w = cjb * D * SHARD
        loop_cm = tc.For_i(0, reps, 1) if reps else None
        if loop_cm is not None:
            loop_cm.__enter__()
        for c in range(n_chunk):
            Pt = pp.tile([P, chunk_w], F32, tag="P", name="Pt")
            CDT = {"r": F32R, "f": F32, "b": mybir.dt.bfloat16, "h": mybir.dt.float16}[cdt]
            Ct = cp.tile([P, chunk_w], CDT, tag="C", name="Ct")
            if "t" not in parts:  # bench-only: keep readers legal
                nc.vector.memset(Pt, 0.0)
            if "e" not in parts and "m" in parts:
                nc.vector.memset(Ct, 0.0)
            for jl in range(cjb):
                jb = cjb * c + jl
                for d in range(D):
                    off = (jl * D + d) * SHARD
                    eng = nc.vector
                    if "t" in parts:
                        eng.tensor_scalar_mul(
                            Pt[:, off : off + SHARD],
                            Zrep[:, SHARD * d : SHARD * (d + 1)],
                            Xws[:, D * jb + d : D * jb + d + 1],
                        )
            if "e" in parts:
                nc.scalar.activation(Ct, Pt, AF.Exp, bias=zb)
            for jl in range(cjb):
                jb = cjb * c + jl
                for d in range(D):
                    off = (jl * D + d) * SHARD
                    if "m" in parts:
                        nc.tensor.matmul(
                            red[d],
                            W6[:, 6 * jb + 2 * d : 6 * jb + 2 * d + 2],
                            Ct[:, off : off + SHARD],
                            start=(jb == 0),
                            stop=(jb == JB - 1),
                        )

        if loop_cm is not None:
            loop_cm.__exit__(None, None, None)

        # ---------- leave-one-out correction + ratio (T-layout, [3,512]) ----------
        t1 = sb.tile([D, SHARD], F32)
        nc.vector.tensor_mul(t1, ZwT, XwTs)
        nhx = sb.tile([D, SHARD], F32)
        nc.vector.tensor_scalar_mul(nhx, XwTs, -0.5)
        t2 = sb.tile([D, SHARD], F32)
        nc.vector.tensor_mul(t2, nhx, XwTs)
        t3 = sb.tile([D, SHARD], F32)  # Zw*Xw - Xw^2/2
        nc.vector.tensor_add(t3, t2, t1)
        cT = sb.tile([D, SHARD], F32)
        nc.scalar.activation(cT, t3, AF.Exp, bias=zb[0:D, :], scale=invh2[0:D, :])
        cY = sb.tile([D, SHARD], F32)
        nc.vector.tensor_mul(cY, cT, YTs)
        # engine ops can't address partition bases 1/2, so gather the PSUM
        # rows into [3,512] tiles via PSUM->SBUF copies + one SBUF DMA per row
        # (a single DMA per consumer keeps every op at <=2 sync waits).
        # spread the three PSUM->SBUF reduction copies across engines so the
        # tail isn't a serial DVE chain
        S6 = sb.tile([2, D * SHARD], F32)
        nc.vector.tensor_copy(S6[:, 0:SHARD], red[0])
        nc.scalar.activation(S6[:, SHARD : 2 * SHARD], red[1], AF.Copy, bias=0.0)
        nc.vector.tensor_copy(S6[:, 2 * SHARD : 3 * SHARD], red[2])
        # the two gather DMAs overlap on separate HWDGE queues (SP + ACT)
        SnT = sb.tile([D, SHARD], F32)
        SdT = sb.tile([D, SHARD], F32)
        nc.sync.dma_start(SnT, S6[0:1, :])
        nc.scalar.dma_start(SdT, S6[1:2, :])
        # numerator subtract on GpSimd in parallel with the denominator
        # subtract + reciprocal chain on DVE
        numT = sb.tile([D, SHARD], F32)
        nc.gpsimd.tensor_sub(numT, SnT, cY)
        denT = sb.tile([D, SHARD], F32)
        nc.vector.tensor_sub(denT, SdT, cT)
        rT = sb.tile([D, SHARD], F32)
        nc.vector.reciprocal(rT, denT)
        oT = sb.tile([D, SHARD], F32)
        nc.vector.tensor_mul(oT, numT, rT)
        nc.sync.dma_start(d_outT, oT)

    nc.compile()
    return nc


def _get_program() -> bass.Bass:
    if "nc" not in _CACHE:
        _CACHE["nc"] = _build_program()
    return _CACHE["nc"]


def _get_exec():
    """AOT-compiled fast-dispatch executable for the 8-core program.

    run_bass_kernel_spmd rebuilds its jax.jit(shard_map(...)) closure on
    every call, so each call pays full retrace + lowering (~200ms on this
    1-cpu host). The device roundtrip itself is a fixed ~70-85ms (axon
    tunnel latency, payload-insensitive), so caching one AOT-compiled
    callable keyed on the (input-independent) program brings the warm call
    down to a single tunnel roundtrip.
    """
    if "exec" in _CACHE:
        return _CACHE["exec"]

    import jax
    from jax.sharding import Mesh, PartitionSpec
    from jax.experimental.shard_map import shard_map
    from concourse.bass2jax import (
        _bass_exec_p,
        partition_id_tensor,
        install_neuronx_cc_hook,
        fast_dispatch_compile,
    )

    nc = _get_program()
    install_neuronx_cc_hook()
    partition_name = nc.partition_id_tensor.name if nc.partition_id_tensor else None
    in_names, out_names, out_avals = [], [], []
    for alloc in nc.m.functions[0].allocations:
        if not isinstance(alloc, mybir.MemoryLocationSet):
            continue
        name = alloc.memorylocations[0].name
        if alloc.kind == "ExternalInput":
            if name != partition_name:
                in_names.append(name)
        elif alloc.kind == "ExternalOutput":
            out_names.append(name)
            out_avals.append(
                jax.core.ShapedArray(
                    tuple(alloc.tensor_shape), mybir.dt.np(alloc.dtype)
                )
            )
    n_params = len(in_names)
    in_names_all = in_names + out_names
    if partition_name is not None:
        in_names_all.append(partition_name)
    donate = tuple(range(n_params, n_params + len(out_names)))

    def _body(*args):
        operands = list(args)
        if partition_name is not None:
            operands.append(partition_id_tensor())
        return tuple(
            _bass_exec_p.bind(
                *operands,
                out_avals=tuple(out_avals),
                in_names=tuple(in_names_all),
                out_names=tuple(out_names),
                lowering_input_output_aliases=(),
                sim_require_finite=True,
                sim_require_nnan=True,
                nc=nc,
            )
        )

    devices = jax.devices()[:NCORES]
    mesh = Mesh(np.asarray(devices), ("core",))
    specs = (PartitionSpec("core"),)
    # global (concat-along-axis-0) example shapes for lowering
    shape_of = {"c4": (4, C4_W), "c3": (D, C3_W), "Yj": (P, JB * D)}
    example_in = [
        np.zeros((NCORES * shape_of[n][0], *shape_of[n][1:]), np.float32)
        for n in in_names
    ]
    example_out = [
        np.zeros((NCORES * a.shape[0], *a.shape[1:]), a.dtype) for a in out_avals
    ]
    compiled = fast_dispatch_compile(
        lambda: jax.jit(
            shard_map(
                _body,
                mesh=mesh,
                in_specs=specs * (n_params + len(out_names)),
                out_specs=specs * len(out_names),
                check_rep=False,
            ),
            donate_argnums=donate,
            keep_unused=True,
        )
        .lower(*example_in, *example_out)
        .compile()
    )
    out_shapes = [
        (NCORES * a.shape[0], *a.shape[1:]) for a in out_avals
    ]
    out_dtypes = [a.dtype for a in out_avals]
    _CACHE["exec"] = (compiled, in_names, out_shapes, out_dtypes)
    return _CACHE["exec"]


def _in_maps(x, train_X, Y, W1, W2, h):
    Yj = np.ascontiguousarray(
        Y.reshape(JB, P, D).transpose(1, 0, 2).reshape(P, JB * D)
    )
    tXT = train_X.T  # [4, N]
    sel = np.zeros((D, D * P), np.float32)
    for d in range(D):
        sel[d, P * d : P * (d + 1)] = 1.0
    maps = []
    for m in range(NCORES):
        sl = slice(SHARD * m, SHARD * (m + 1))
        c4 = np.empty((4, C4_W), np.float32)
        c4[:, 0:SHARD] = x[sl].T
        c4[:, SHARD : SHARD + D] = W1.T
        c4[:, SHARD + D : SHARD + D + N] = tXT
        c4[:, SHARD + D + N :] = train_X[sl].T
        c3 = np.zeros((D, C3_W), np.float32)
        c3[:, 0:D] = W2.T
        c3[:, D : D + SHARD] = Y[sl].T
        c3[:, D + SHARD : D + SHARD + D * P] = sel
        c3[0, D + SHARD + D * P : D + SHARD + D * P + D * D] = W2.reshape(-1)
        c3[0, D + SHARD + D * P + D * D] = np.float32(h)
        maps.append({"c4": c4, "c3": c3, "Yj": Yj})
    return maps


def _in_concat(x, train_X, Y, W1, W2, h):
    """Pack per-core inputs directly into the global concat-axis-0 blobs."""
    tXT = train_X.T  # [4, N]
    xT = x.T
    YT = Y.T
    c4g = np.empty((NCORES * 4, C4_W), np.float32)
    c3g = np.zeros((NCORES * D, C3_W), np.float32)
    Yjg = np.empty((NCORES * P, JB * D), np.float32)
    Yj = np.ascontiguousarray(
        Y.reshape(JB, P, D).transpose(1, 0, 2).reshape(P, JB * D)
    )
    sel = np.zeros((D, D * P), np.float32)
    for d in range(D):
        sel[d, P * d : P * (d + 1)] = 1.0
    W1T = W1.T
    W2T = W2.T
    W2f = W2.reshape(-1)
    for m in range(NCORES):
        sl = slice(SHARD * m, SHARD * (m + 1))
        c4 = c4g[4 * m : 4 * (m + 1)]
        c4[:, 0:SHARD] = xT[:, sl]
        c4[:, SHARD : SHARD + D] = W1T
        c4[:, SHARD + D : SHARD + D + N] = tXT
        c4[:, SHARD + D + N :] = tXT[:, sl]
        c3 = c3g[D * m : D * (m + 1)]
        c3[:, 0:D] = W2T
        c3[:, D : D + SHARD] = YT[:, sl]
        c3[:, D + SHARD : D + SHARD + D * P] = sel
        c3[0, D + SHARD + D * P : D + SHARD + D * P + D * D] = W2f
        c3[0, D + SHARD + D * P + D * D] = np.float32(h)
        Yjg[P * m : P * (m + 1)] = Yj
    return {"c4": c4g, "c3": c3g, "Yj": Yjg}


def kernel(x, train_X, Y, W1, W2, h, **run_kwargs):
    x = np.asarray(x, np.float32)
    train_X = np.asarray(train_X, np.float32)
    Y = np.asarray(Y, np.float32)
    W1 = np.asarray(W1, np.float32)
    W2 = np.asarray(W2, np.float32)

    if run_kwargs:
        # trace / debug path: the original (slow) per-call spmd runner
        nc = _get_program()
        maps = _in_maps(x, train_X, Y, W1, W2, h)
        rr = run_bass_kernel_spmd(nc, maps, list(range(NCORES)), **run_kwargs)
        out = np.concatenate(
            [np.asarray(rr.results[m]["outT"]).T for m in range(NCORES)], axis=0
        )
        kernel.last_results = rr
        return np.ascontiguousarray(out, np.float32)

    if not _CACHE.get("exec_failed"):
        try:
            compiled, in_names, out_shapes, out_dtypes = _get_exec()
        except Exception:
            _CACHE["exec_failed"] = True
    if _CACHE.get("exec_failed"):
        nc = _get_program()
        maps = _in_maps(x, train_X, Y, W1, W2, h)
        rr = run_bass_kernel_spmd(nc, maps, list(range(NCORES)))
        out = np.concatenate(
            [np.asarray(rr.results[m]["outT"]).T for m in range(NCORES)], axis=0
        )
        return np.ascontiguousarray(out, np.float32)

    blobs = _in_concat(x, train_X, Y, W1, W2, h)
    args = [blobs[n] for n in in_names]
    zeros = [np.zeros(s, dt) for s, dt in zip(out_shapes, out_dtypes)]
    out_arrs = compiled(*args, *zeros)
    o = np.asarray(out_arrs[0])  # [NCORES*D, SHARD]
    out = np.empty((N, D), np.float32)
    for m in range(NCORES):
        out[SHARD * m : SHARD * (m + 1)] = o[D * m : D * (m + 1)].T
    return out



# revision 36
# speedup vs baseline: 1.0562x; 1.0562x over previous
"""Trainium2 Bass kernel for leave-one-out Nadaraya-Watson regression
(nn_Net_7610682049228, retrieval_knn).

Math
----
Zw = relu(x @ W1.T) @ W2.T          [N, 3]
Xw = relu(train_X @ W1.T) @ W2.T    [N, 3]
K[i,j,d] = exp(-((Xw[j,d]-Zw[i,d])/h)^2 / 2), diagonal i==j masked out
out[i,d] = sum_j K*Y / sum_j K

Kernel factorization (the key trick):
  K[i,j,d] = G[j,d] * H[i,d] * C[i,j,d]
    G[j,d] = exp(-Xw[j,d]^2 / 2h^2)        (O(N) precompute)
    H[i,d] = exp(-Zw[i,d]^2 / 2h^2)        (cancels in the ratio!)
    C[i,j,d] = exp(Zw[i,d]*Xw[j,d] / h^2)  (rank-1 exponent)
  out[i,d] = (sum_j C*G*Y - c_i*Y_i) / (sum_j C*G - c_i)
    with the leave-one-out correction c[i,d] = exp((Zw*Xw - Xw^2/2)/h^2)|_{j=i}.

So the only O(N^2) work is: a rank-1 outer product (DVE tensor_scalar with a
per-partition scalar), one big Exp pass (ACT engine - the throughput floor),
and [G*Y | G]-weighted column reductions (PE matmuls accumulating in PSUM).

Sharding: data-parallel over query rows i; core m handles i in
[512m, 512m+512). j lives on SBUF partitions (32 blocks of 128), the 512
i-columns of the shard live on the free dim. No cross-core communication.

All input-dependent scalars (h, W2) are consumed as tensors, so the compiled
program is input-independent and built/compiled once per process.

Execution layer: on this axon-tunneled setup every device roundtrip has a
fixed ~70-85ms turnaround (payload- and device-count-insensitive; the
request leaves in ~2ms and the line is silent until the response), so the
warm-call wall time is one roundtrip plus host packing. kernel() therefore
AOT-compiles one fast-dispatch executable for the 8-core shard_map program
(cached in _CACHE) and each call is: pack inputs (~1ms) -> dispatch
(~2-9ms, upload overlaps the roundtrip) -> one await+fetch (~70ms).
run_bass_kernel_spmd, which rebuilds its jax.jit closure per call
(~200ms retrace), is kept as the trace/debug and fallback path.
"""

import numpy as np
from contextlib import ExitStack

import concourse.bacc as bacc
import concourse.bass as bass
import concourse.mybir as mybir
import concourse.tile as tile
from concourse.bass_utils import run_bass_kernel_spmd

F32 = mybir.dt.float32
AF = mybir.ActivationFunctionType
OP = mybir.AluOpType

N = 4096
NCORES = 8
SHARD = N // NCORES          # 512 query rows per core
P = 128                      # SBUF partitions
JB = N // P                  # 32 j-blocks
D = 3                        # output dims
JB_PER_CHUNK = 2             # j-blocks fused into one ACT Exp instruction
NCHUNK = JB // JB_PER_CHUNK  # 16
CHUNK_W = JB_PER_CHUNK * D * SHARD  # 3072 free elements per chunk
C4_W = SHARD + D + N + SHARD           # [xTs | W1T | tXT | tXTs]
C3_W = D + SHARD + D * P + D * D + 1   # [W2T | YTs | sel | W2f | h]

_CACHE = {}


def _build_program(reps: int = 0, parts: str = "tem", cdt: str = "r", cjb: int = JB_PER_CHUNK) -> bass.Bass:
    # Bacc (not raw Bass): its compile() pass legalizes multi-wait
    # instructions for walrus, which allows only 1-2 sync waits per op.
    # reps > 0 wraps the main O(N^2) loop in a hardware For_i that repeats it
    # `reps` times — used only for wall-clock calibration benchmarks.
    # parts: which main-loop stages to emit (t=tensor_scalar, e=exp, m=matmul)
    # — benchmarking aid, always "tem" for real runs.
    nc = bacc.Bacc("TRN2", target_bir_lowering=False, debug=False)

    # --- DRAM I/O (per-core shapes; host preps layouts/slices) ---
    d_c4 = nc.dram_tensor("c4", (4, C4_W), F32, kind="ExternalInput").ap()
    d_c3 = nc.dram_tensor("c3", (D, C3_W), F32, kind="ExternalInput").ap()
    d_Yj = nc.dram_tensor("Yj", (P, JB * D), F32, kind="ExternalInput").ap()
    d_outT = nc.dram_tensor("outT", (D, SHARD), F32, kind="ExternalOutput").ap()

    with tile.TileContext(nc) as tc, ExitStack() as ctx:
        sb = ctx.enter_context(tc.tile_pool(name="sb", bufs=1))
        pp = ctx.enter_context(tc.tile_pool(name="pp", bufs=3))
        cp = ctx.enter_context(tc.tile_pool(name="cp", bufs=3))
        ps = ctx.enter_context(tc.tile_pool(name="ps", bufs=1, space="PSUM"))
        pr = ctx.enter_context(tc.tile_pool(name="pr", bufs=1, space="PSUM"))
        # Setup matmul chains get their own PSUM tiles so the chains overlap
        # (one shared scratch serialized the whole ~23us setup through
        # PE->copy->PE round trips). Banks: z, x, bj, b + 3x red.
        PS_z = ps.tile([D, SHARD], F32, tag="ps_z", name="PS_z")
        PS_x = ps.tile([D, SHARD], F32, tag="ps_x", name="PS_x")
        PS_bj = ps.tile([P, JB * D], F32, tag="ps_bj", name="PS_bj")
        PS_b = ps.tile([P, 1 + D * D], F32, tag="ps_b", name="PS_b")

        # ---------- load inputs (HWDGE; Bacc legalizes multi-wait consumers)
        # Host packs the small tensors into combo blobs to minimize DMA
        # instruction count (each DMA costs ~descriptor-count in setup time).
        def load(dram_ap, shape, name):
            t = sb.tile(shape, F32, name=name)
            nc.sync.dma_start(t, dram_ap)
            return t

        # c4 split: the 8KB [xTs|W1T] head lands first so the query-MLP
        # (the long pole of setup: MM1_z -> relu -> MM2_z -> Zrep) isn't
        # gated on the 82KB tail. c3/Yj issue from the Activation HWDGE
        # queue so the four input DMAs overlap across both queues.
        HEAD = SHARD + D
        c4 = sb.tile([4, C4_W], F32, name="c4")
        nc.sync.dma_start(c4[:, 0:HEAD], d_c4[:, 0:HEAD])
        nc.sync.dma_start(c4[:, HEAD:], d_c4[:, HEAD:])
        xTs = c4[:, 0:SHARD]
        W1T = c4[:, SHARD : SHARD + D]
        tXT = c4[:, HEAD : HEAD + N]
        tXTs = c4[:, HEAD + N : HEAD + N + SHARD]
        c3 = sb.tile([D, C3_W], F32, name="c3")
        nc.scalar.dma_start(c3, d_c3)
        W2T = c3[:, 0:D]
        YTs = c3[:, D : D + SHARD]
        sel = c3[:, D + SHARD : D + SHARD + D * P]
        W2f = c3[0:1, D + SHARD + D * P : D + SHARD + D * P + D * D]
        h_sb = c3[0:1, D + SHARD + D * P + D * D : D + SHARD + D * P + D * D + 1]
        Yj = sb.tile([P, JB * D], F32, name="Yj")
        nc.scalar.dma_start(Yj, d_Yj)

        ones = sb.tile([1, P], F32)
        nc.vector.memset(ones, 1.0)
        zb = sb.tile([P, 1], F32)  # zero bias for activations
        nc.vector.memset(zb, 0.0)

        # ---------- broadcast scalars: 1/h^2 and W2 across partitions ----------
        hsq = sb.tile([1, 1], F32)
        nc.vector.tensor_mul(hsq, h_sb, h_sb)
        hinv = sb.tile([1, 1], F32)
        nc.vector.reciprocal(hinv, hsq)
        W2h = sb.tile([1, 1 + D * D], F32)  # [1/h^2, W2 row-major]
        nc.vector.tensor_copy(W2h[:, 0:1], hinv)
        nc.vector.tensor_copy(W2h[:, 1:], W2f)
        # broadcast gets its own PSUM tile so the bc copy doesn't create a
        # false WAR dependency that delays the j-MLP matmuls
        nc.tensor.matmul(PS_b, ones, W2h, start=True, stop=True)
        bc = sb.tile([P, 1 + D * D], F32)
        nc.vector.tensor_copy(bc, PS_b)
        invh2 = bc[:, 0:1]

        def w2col(d, m):  # W2[d,m] broadcast per-partition
            return bc[:, 1 + D * d + m : 2 + D * d + m]

        nh = sb.tile([P, 1], F32)  # -1/(2 h^2), ACT scale for G
        nc.vector.tensor_scalar_mul(nh, invh2, -0.5)

        # fp32r: PE streams it at 1 col/cycle when the moving dim >= 256
        # (plain fp32 matmul is 4x slower), at slightly reduced precision.
        # walrus requires fp32r matmul operands to be *produced* as fp32r,
        # so the hot-loop tiles (C, W6) are allocated fp32r and rounded on
        # write by ACT/DVE; the tiny setup matmuls stay plain fp32.
        F32R = mybir.dt.float32r

        # ---------- MLPs. Critical-path chains first: the query MLP
        # (-> ZwT -> Zrep) and the j-layout MLP (-> Xws/W6) gate the main
        # loop; the XwTs chain feeds only the epilogue and is emitted last.
        nc.tensor.matmul(PS_z, W1T, xTs, start=True, stop=True)
        for jb in range(JB):
            nc.tensor.matmul(
                PS_bj[:, D * jb : D * (jb + 1)],
                tXT[:, P * jb : P * (jb + 1)],
                W1T,
                start=True,
                stop=True,
            )
        hidz = sb.tile([D, SHARD], F32, name="hidz")
        nc.scalar.activation(hidz, PS_z, AF.Relu, bias=zb[0:D, :])
        h1j = sb.tile([P, JB * D], F32)
        nc.scalar.activation(h1j, PS_bj[:, 0 : JB * D], AF.Relu, bias=zb)
        nc.tensor.matmul(PS_z, W2T, hidz, start=True, stop=True)
        ZwT = sb.tile([D, SHARD], F32, name="mlpTz")  # unscaled
        # Identity-activation copy keeps this off the congested DVE queue
        nc.scalar.activation(ZwT, PS_z, AF.Copy, bias=0.0)
        # layer 2 on DVE with per-partition W2 scalars
        h1r = h1j.rearrange("p (a m) -> p a m", m=D)
        Xwj = sb.tile([P, JB * D], F32)
        Xwr = Xwj.rearrange("p (a d) -> p a d", d=D)
        for d in range(D):
            acc0 = sb.tile([P, JB], F32, tag="l2a", name="acc0")
            nc.vector.tensor_scalar_mul(acc0, h1r[:, :, 0], w2col(d, 0))
            acc1 = sb.tile([P, JB], F32, tag="l2b", name="acc1")
            nc.vector.scalar_tensor_tensor(
                acc1, h1r[:, :, 1], w2col(d, 1), acc0, OP.mult, OP.add
            )
            nc.vector.scalar_tensor_tensor(
                Xwr[:, :, d], h1r[:, :, 2], w2col(d, 2), acc1, OP.mult, OP.add
            )
        # Xw scaled by 1/h^2: the per-partition scalar for the rank-1 products
        Xws = sb.tile([P, JB * D], F32)
        nc.vector.tensor_scalar_mul(Xws, Xwj, invh2)

        # ---------- G, G*Y -> interleaved matmul weights W6 ----------
        sq = sb.tile([P, JB * D], F32)
        nc.vector.tensor_mul(sq, Xwj, Xwj)
        W6 = sb.tile(
            [P, JB * D * 2],
            {"r": F32R, "f": F32, "b": mybir.dt.bfloat16, "h": mybir.dt.float16}[cdt],
        )
        W6r = W6.rearrange("p (a t) -> p a t", t=2)
        # write G and G*Y straight into the interleaved weight tile (strided
        # outputs) instead of materializing Gj/GYj and copying them over
        nc.scalar.activation(W6r[:, :, 1], sq, AF.Exp, bias=zb, scale=nh)
        nc.vector.tensor_mul(W6r[:, :, 0], W6r[:, :, 1], Yj)

        # ---------- Zw replicated across partitions: [128, 3*512] ----------
        # Linearize ZwT [3,512] onto partition 0 with one SBUF->SBUF DMA,
        # then a single GpSimd partition_broadcast fans it out to all 128
        # partitions. This replaces three 2.4us contraction-3 PE matmuls
        # (the old one-hot sel trick) with ~2us on otherwise-idle engines.
        Zw3 = sb.tile([1, D * SHARD], F32, name="Zw3")
        nc.sync.dma_start(Zw3, ZwT)
        Zrep = sb.tile([P, D * SHARD], F32)
        nc.gpsimd.partition_broadcast(Zrep, Zw3)

        # deferred XwTs chain (feeds only the leave-one-out epilogue)
        nc.tensor.matmul(PS_x, W1T, tXTs, start=True, stop=True)
        hidx = sb.tile([D, SHARD], F32, name="hidx")
        nc.scalar.activation(hidx, PS_x, AF.Relu, bias=zb[0:D, :])
        nc.tensor.matmul(PS_x, W2T, hidx, start=True, stop=True)
        XwTs = sb.tile([D, SHARD], F32, name="mlpTx")  # unscaled
        nc.vector.tensor_copy(XwTs, PS_x)

        # ---------- main O(N^2) loop ----------
        red = [
            pr.tile([2, SHARD], F32, tag=f"red{d}", name=f"red{d}") for d in range(D)
        ]
        if "m" not in parts:  # bench-only: keep epilogue readers legal
            for d in range(D):
                nc.vector.memset(red[d], 1.0)
        n_chunk = JB // cjb
        chunk_w = cjb * D * SHARD
        loop_cm = tc.For_i(0, reps, 1) if reps else None
        if loop_cm is not None:
            loop_cm.__enter__()
        for c in range(n_chunk):
            Pt = pp.tile([P, chunk_w], F32, tag="P", name="Pt")
            CDT = {"r": F32R, "f": F32, "b": mybir.dt.bfloat16, "h": mybir.dt.float16}[cdt]
            Ct = cp.tile([P, chunk_w], CDT, tag="C", name="Ct")
            if "t" not in parts:  # bench-only: keep readers legal
                nc.vector.memset(Pt, 0.0)
            if "e" not in parts and "m" in parts:
                nc.vector.memset(Ct, 0.0)
            for jl in range(cjb):
                jb = cjb * c + jl
                for d in range(D):
                    off = (jl * D + d) * SHARD
                    eng = nc.vector
                    if "t" in parts:
                        eng.tensor_scalar_mul(
                            Pt[:, off : off + SHARD],
                            Zrep[:, SHARD * d : SHARD * (d + 1)],
                            Xws[:, D * jb + d : D * jb + d + 1],
                        )
            if "e" in parts:
                nc.scalar.activation(Ct, Pt, AF.Exp, bias=zb)
            for jl in range(cjb):
                jb = cjb * c + jl
                for d in range(D):
                    off = (jl * D + d) * SHARD
                    if "m" in parts:
                        nc.tensor.matmul(
                            red[d],
                            W6[:, 6 * jb + 2 * d : 6 * jb + 2 * d + 2],
                            Ct[:, off : off + SHARD],
                            start=(jb == 0),
                            stop=(jb == JB - 1),
                        )

        if loop_cm is not None:
            loop_cm.__exit__(None, None, None)

        # ---------- leave-one-out correction + ratio (T-layout, [3,512]) ----------
        t1 = sb.tile([D, SHARD], F32)
        nc.vector.tensor_mul(t1, ZwT, XwTs)
        nhx = sb.tile([D, SHARD], F32)
        nc.vector.tensor_scalar_mul(nhx, XwTs, -0.5)
        t2 = sb.tile([D, SHARD], F32)
        nc.vector.tensor_mul(t2, nhx, XwTs)
        t3 = sb.tile([D, SHARD], F32)  # Zw*Xw - Xw^2/2
        nc.vector.tensor_add(t3, t2, t1)
        cT = sb.tile([D, SHARD], F32)
        nc.scalar.activation(cT, t3, AF.Exp, bias=zb[0:D, :], scale=invh2[0:D, :])
        cY = sb.tile([D, SHARD], F32)
        nc.vector.tensor_mul(cY, cT, YTs)
        # engine ops can't address partition bases 1/2, so gather the PSUM
        # rows into [3,512] tiles via PSUM->SBUF copies + one SBUF DMA per row
        # (a single DMA per consumer keeps every op at <=2 sync waits).
        # spread the three PSUM->SBUF reduction copies across engines so the
        # tail isn't a serial DVE chain
        S6 = sb.tile([2, D * SHARD], F32)
        nc.vector.tensor_copy(S6[:, 0:SHARD], red[0])
        nc.scalar.activation(S6[:, SHARD : 2 * SHARD], red[1], AF.Copy, bias=0.0)
        nc.vector.tensor_copy(S6[:, 2 * SHARD : 3 * SHARD], red[2])
        # the two gather DMAs overlap on separate HWDGE queues (SP + ACT)
        SnT = sb.tile([D, SHARD], F32)
        SdT = sb.tile([D, SHARD], F32)
        nc.sync.dma_start(SnT, S6[0:1, :])
        nc.scalar.dma_start(SdT, S6[1:2, :])
        # numerator subtract on GpSimd in parallel with the denominator
        # subtract + reciprocal chain on DVE
        numT = sb.tile([D, SHARD], F32)
        nc.gpsimd.tensor_sub(numT, SnT, cY)
        denT = sb.tile([D, SHARD], F32)
        nc.vector.tensor_sub(denT, SdT, cT)
        rT = sb.tile([D, SHARD], F32)
        nc.vector.reciprocal(rT, denT)
        oT = sb.tile([D, SHARD], F32)
        nc.vector.tensor_mul(oT, numT, rT)
        nc.sync.dma_start(d_outT, oT)

    nc.compile()
    return nc


def _get_program() -> bass.Bass:
    if "nc" not in _CACHE:
        _CACHE["nc"] = _build_program()
    return _CACHE["nc"]


def _get_exec():
    """AOT-compiled fast-dispatch executable for the 8-core program.

    run_bass_kernel_spmd rebuilds its jax.jit(shard_map(...)) closure on
    every call, so each call pays full retrace + lowering (~200ms on this
    1-cpu host). The device roundtrip itself is a fixed ~70-85ms (axon
    tunnel latency, payload-insensitive), so caching one AOT-compiled
    callable keyed on the (input-independent) program brings the warm call
    down to a single tunnel roundtrip.
    """
    if "exec" in _CACHE:
        return _CACHE["exec"]

    import jax
    from jax.sharding import Mesh, PartitionSpec
    from jax.experimental.shard_map import shard_map
    from concourse.bass2jax import (
        _bass_exec_p,
        partition_id_tensor,
        install_neuronx_cc_hook,
        fast_dispatch_compile,
    )

    nc = _get_program()
    install_neuronx_cc_hook()
    partition_name = nc.partition_id_tensor.name if nc.partition_id_tensor else None
    in_names, out_names, out_avals = [], [], []
    for alloc in nc.m.functions[0].allocations:
        if not isinstance(alloc, mybir.MemoryLocationSet):
            continue
        name = alloc.memorylocations[0].name
        if alloc.kind == "ExternalInput":
            if name != partition_name:
                in_names.append(name)
        elif alloc.kind == "ExternalOutput":
            out_names.append(name)
            out_avals.append(
                jax.core.ShapedArray(
                    tuple(alloc.tensor_shape), mybir.dt.np(alloc.dtype)
                )
            )
    n_params = len(in_names)
    in_names_all = in_names + out_names
    if partition_name is not None:
        in_names_all.append(partition_name)
    donate = tuple(range(n_params, n_params + len(out_names)))

    def _body(*args):
        operands = list(args)
        if partition_name is not None:
            operands.append(partition_id_tensor())
        return tuple(
            _bass_exec_p.bind(
                *operands,
                out_avals=tuple(out_avals),
                in_names=tuple(in_names_all),
                out_names=tuple(out_names),
                lowering_input_output_aliases=(),
                sim_require_finite=True,
                sim_require_nnan=True,
                nc=nc,
            )
        )

    devices = jax.devices()[:NCORES]
    mesh = Mesh(np.asarray(devices), ("core",))
    specs = (PartitionSpec("core"),)
    # global (concat-along-axis-0) example shapes for lowering
    shape_of = {"c4": (4, C4_W), "c3": (D, C3_W), "Yj": (P, JB * D)}
    example_in = [
        np.zeros((NCORES * shape_of[n][0], *shape_of[n][1:]), np.float32)
        for n in in_names
    ]
    example_out = [
        np.zeros((NCORES * a.shape[0], *a.shape[1:]), a.dtype) for a in out_avals
    ]
    compiled = fast_dispatch_compile(
        lambda: jax.jit(
            shard_map(
                _body,
                mesh=mesh,
                in_specs=specs * (n_params + len(out_names)),
                out_specs=specs * len(out_names),
                check_rep=False,
            ),
            donate_argnums=donate,
            keep_unused=True,
        )
        .lower(*example_in, *example_out)
        .compile()
    )
    out_shapes = [
        (NCORES * a.shape[0], *a.shape[1:]) for a in out_avals
    ]
    out_dtypes = [a.dtype for a in out_avals]
    _CACHE["exec"] = (compiled, in_names, out_shapes, out_dtypes)
    return _CACHE["exec"]


def _in_maps(x, train_X, Y, W1, W2, h):
    Yj = np.ascontiguousarray(
        Y.reshape(JB, P, D).transpose(1, 0, 2).reshape(P, JB * D)
    )
    tXT = train_X.T  # [4, N]
    sel = np.zeros((D, D * P), np.float32)
    for d in range(D):
        sel[d, P * d : P * (d + 1)] = 1.0
    maps = []
    for m in range(NCORES):
        sl = slice(SHARD * m, SHARD * (m + 1))
        c4 = np.empty((4, C4_W), np.float32)
        c4[:, 0:SHARD] = x[sl].T
        c4[:, SHARD : SHARD + D] = W1.T
        c4[:, SHARD + D : SHARD + D + N] = tXT
        c4[:, SHARD + D + N :] = train_X[sl].T
        c3 = np.zeros((D, C3_W), np.float32)
        c3[:, 0:D] = W2.T
        c3[:, D : D + SHARD] = Y[sl].T
        c3[:, D + SHARD : D + SHARD + D * P] = sel
        c3[0, D + SHARD + D * P : D + SHARD + D * P + D * D] = W2.reshape(-1)
        c3[0, D + SHARD + D * P + D * D] = np.float32(h)
        maps.append({"c4": c4, "c3": c3, "Yj": Yj})
    return maps


def _in_concat(x, train_X, Y, W1, W2, h):
    """Pack per-core inputs directly into the global concat-axis-0 blobs."""
    tXT = train_X.T  # [4, N]
    xT = x.T
    YT = Y.T
    c4g = np.empty((NCORES * 4, C4_W), np.float32)
    c3g = np.zeros((NCORES * D, C3_W), np.float32)
    Yjg = np.empty((NCORES * P, JB * D), np.float32)
    Yj = np.ascontiguousarray(
        Y.reshape(JB, P, D).transpose(1, 0, 2).reshape(P, JB * D)
    )
    sel = np.zeros((D, D * P), np.float32)
    for d in range(D):
        sel[d, P * d : P * (d + 1)] = 1.0
    W1T = W1.T
    W2T = W2.T
    W2f = W2.reshape(-1)
    for m in range(NCORES):
        sl = slice(SHARD * m, SHARD * (m + 1))
        c4 = c4g[4 * m : 4 * (m + 1)]
        c4[:, 0:SHARD] = xT[:, sl]
        c4[:, SHARD : SHARD + D] = W1T
        c4[:, SHARD + D : SHARD + D + N] = tXT
        c4[:, SHARD + D + N :] = tXT[:, sl]
        c3 = c3g[D * m : D * (m + 1)]
        c3[:, 0:D] = W2T
        c3[:, D : D + SHARD] = YT[:, sl]
        c3[:, D + SHARD : D + SHARD + D * P] = sel
        c3[0, D + SHARD + D * P : D + SHARD + D * P + D * D] = W2f
        c3[0, D + SHARD + D * P + D * D] = np.float32(h)
        Yjg[P * m : P * (m + 1)] = Yj
    return {"c4": c4g, "c3": c3g, "Yj": Yjg}


def kernel(x, train_X, Y, W1, W2, h, **run_kwargs):
    x = np.asarray(x, np.float32)
    train_X = np.asarray(train_X, np.float32)
    Y = np.asarray(Y, np.float32)
    W1 = np.asarray(W1, np.float32)
    W2 = np.asarray(W2, np.float32)

    if run_kwargs:
        # trace / debug path: the original (slow) per-call spmd runner
        nc = _get_program()
        maps = _in_maps(x, train_X, Y, W1, W2, h)
        rr = run_bass_kernel_spmd(nc, maps, list(range(NCORES)), **run_kwargs)
        out = np.concatenate(
            [np.asarray(rr.results[m]["outT"]).T for m in range(NCORES)], axis=0
        )
        kernel.last_results = rr
        return np.ascontiguousarray(out, np.float32)

    if not _CACHE.get("exec_failed"):
        try:
            compiled, in_names, out_shapes, out_dtypes = _get_exec()
        except Exception:
            _CACHE["exec_failed"] = True
    if _CACHE.get("exec_failed"):
        nc = _get_program()
        maps = _in_maps(x, train_X, Y, W1, W2, h)
        rr = run_bass_kernel_spmd(nc, maps, list(range(NCORES)))
        out = np.concatenate(
            [np.asarray(rr.results[m]["outT"]).T for m in range(NCORES)], axis=0
        )
        return np.ascontiguousarray(out, np.float32)

    blobs = _in_concat(x, train_X, Y, W1, W2, h)
    args = [blobs[n] for n in in_names]
    zeros = [np.zeros(s, dt) for s, dt in zip(out_shapes, out_dtypes)]
    out_arrs = compiled(*args, *zeros)
    o = np.asarray(out_arrs[0])  # [NCORES*D, SHARD]
    out = np.empty((N, D), np.float32)
    for m in range(NCORES):
        out[SHARD * m : SHARD * (m + 1)] = o[D * m : D * (m + 1)].T
    return out

